# revision 1
# baseline (speedup 1.0000x reference)
"""Bass/Tile kernel for nn_AttnModule (sparse_attention).

Reference computation (per batch b):
    scores  = pos_emb @ position_fmap[b].T          # [T, L]
    attn    = softmax(scores, axis=-1)              # softmax over L
    context = attn @ origin_fmap[b]                 # [T, H]
    out     = context @ W_gen.T + b_gen             # [T, C]

Sharding: pure data parallel over batch B=64 -> 8 cores x 8 batches.

Dtype strategy: all matmuls in fp16 (1 cycle/row on PE, 2-byte DMA);
pos_emb is shipped as an fp16 hi/lo pair and mm1 accumulates both terms
in fp32 PSUM, which removes the dominant logit-rounding error (softmax
here is extremely peaked: scores ~ N(0, 512) unscaled). Softmax
statistics are fp32 throughout.

Structure: software-pipelined across batches with PIPE_OFFSET so the PE
stream alternates front-half (mm1 + softmax + attn-transpose) of batch
b with back-half (mm2 + ctx-transpose + mm3) of batch b-OFFSET, keeping
PE dense instead of stalling on the per-batch serial chain.

Layout choices (host side prep is free):
  - position_fmap shipped pre-transposed per batch: pfT [B, H, L]
  - pos_emb shipped transposed as hi/lo fp16 pair, T zero-padded to
    TP=112: peT [2, H, TP]
  - W_gen shipped transposed: wgT [H, C]
  - output produced as [B, C, T] and transposed back on host.
"""

import numpy as np

import concourse.mybir as mybir
import concourse.tile as tile
from concourse import bacc
from concourse.bass_utils import run_bass_kernel_spmd
from concourse.masks import make_identity

B, L, H, T, C = 64, 1024, 512, 100, 97
TP = 128
NCORES = 8
BPC = B // NCORES  # batches per core

HT = H // 128  # 4 h-tiles
LT = L // 128  # 8 l-tiles

F32 = mybir.dt.float32
AF = mybir.ActivationFunctionType
AX = mybir.AxisListType
OP = mybir.AluOpType

MM_DT = mybir.dt.float16
NP_DT = np.float16
PE_TERMS = 2  # pos_emb hi/lo pair
PIPE_OFFSET = 1
PHASED = True  # all front-halves (and pf loads) before all back-halves


def build_nc(mm_dt=MM_DT, pe_terms=PE_TERMS, repeats=1, pipe_offset=PIPE_OFFSET, phased=PHASED, part_split=False, of_halves=False, fine_split=False, of_on_act=False, lh_inner=False):
    nc = bacc.Bacc(None, target_bir_lowering=False, debug=False)

    pfT = nc.dram_tensor("pfT", [BPC, 128, HT, L], mm_dt, kind="ExternalInput").ap()
    of = nc.dram_tensor("of", [BPC, 128, LT, H], mm_dt, kind="ExternalInput").ap()
    peT = nc.dram_tensor("peT", [pe_terms, H, TP], mm_dt, kind="ExternalInput").ap()
    wgT = nc.dram_tensor("wgT", [H, C], mm_dt, kind="ExternalInput").ap()
    bg = nc.dram_tensor("bg", [C, 1], F32, kind="ExternalInput").ap()
    outT = nc.dram_tensor("outT", [BPC, C, T], mm_dt, kind="ExternalOutput").ap()

    nbufs = BPC if phased else pipe_offset + 3  # live range of pT/rinv tiles

    with tile.TileContext(nc) as tc:
        with (
            tc.tile_pool(name="consts", bufs=1) as consts,
            tc.tile_pool(name="pf", bufs=8) as pfpool,
            tc.tile_pool(name="ofp", bufs=BPC) as ofpool,
            tc.tile_pool(name="mid", bufs=nbufs) as mid,  # tiles crossing the pipeline seam
            tc.tile_pool(name="work", bufs=3) as work,
            tc.tile_pool(name="ps_scores", bufs=3, space="PSUM") as ps_scores,
            tc.tile_pool(name="ps_tp", bufs=2, space="PSUM") as ps_tp,
            tc.tile_pool(name="ps_ctx", bufs=3, space="PSUM") as ps_ctx,
        ):
            # ---- constants ----
            # peT first (gates mm1(0)), split per term so term 0 lands sooner;
            # wgT/bg (back-phase only) are loaded between the pf and of blocks.
            peT_sb = consts.tile([128, pe_terms, HT, TP], mm_dt)
            peTr = peT.rearrange("e (ht p) t -> p e ht t", p=128)
            for e in range(pe_terms):
                nc.sync.dma_start(peT_sb[:, e], peTr[:, e])
            wgT_sb = consts.tile([128, HT, C], mm_dt)
            bg_sb = consts.tile([C, 1], F32)
            ident = consts.tile([128, 128], mm_dt)
            make_identity(nc, ident)

            def load_of(b):
                # host ships of pre-tiled [128, LT, H]: per-partition data is
                # fully contiguous in DRAM -> 4KB descriptors per half
                of_sb = ofpool.tile([128, LT, H], mm_dt, tag="of")
                if part_split:
                    for i in range(4):
                        nc.sync.dma_start(
                            of_sb[32 * i : 32 * (i + 1)], of[b, 32 * i : 32 * (i + 1)]
                        )
                else:
                    n = 2 if of_halves else (8 if fine_split else 4)
                    q = LT // n
                    eng = nc.scalar if of_on_act else nc.sync
                    for i in range(n):
                        eng.dma_start(
                            of_sb[:, i * q : (i + 1) * q, :], of[b, :, i * q : (i + 1) * q, :]
                        )
                return of_sb

            def front_half(b):
                """mm1 + softmax + attn transpose -> (pT, rinv) tiles."""
                # pf pre-tiled [128, HT, L] host-side; per-ht DMAs: separate
                # HWDGE queue entries parallelize across DMA engines
                pf_sb = pfpool.tile([128, HT, L], mm_dt, tag="pf")
                if part_split:
                    # 4 DMAs over disjoint 32-partition ranges; each partition's
                    # data is one contiguous 8KB DRAM run
                    for i in range(4):
                        nc.sync.dma_start(
                            pf_sb[32 * i : 32 * (i + 1)], pfT[b, 32 * i : 32 * (i + 1)]
                        )
                elif fine_split:
                    for ht in range(HT):
                        for lh in range(2):
                            nc.sync.dma_start(
                                pf_sb[:, ht, lh * 512 : (lh + 1) * 512],
                                pfT[b, :, ht, lh * 512 : (lh + 1) * 512],
                            )
                else:
                    for ht in range(HT):
                        nc.sync.dma_start(pf_sb[:, ht, :], pfT[b, :, ht, :])

                sc_ps = [
                    ps_scores.tile([TP, 512], F32, tag="scores", name=f"sc{lh}")
                    for lh in range(L // 512)
                ]
                if lh_inner:
                    # consecutive matmuls share the stationary operand
                    for e in range(pe_terms):
                        for ht in range(HT):
                            for lh in range(L // 512):
                                nc.tensor.matmul(
                                    sc_ps[lh],
                                    lhsT=peT_sb[:, e, ht, :],
                                    rhs=pf_sb[:, ht, lh * 512 : (lh + 1) * 512],
                                    start=(e == 0 and ht == 0),
                                    stop=(e == pe_terms - 1 and ht == HT - 1),
                                )
                else:
                    for lh in range(L // 512):
                        first = True
                        for e in range(pe_terms):
                            for ht in range(HT):
                                last = e == pe_terms - 1 and ht == HT - 1
                                nc.tensor.matmul(
                                    sc_ps[lh],
                                    lhsT=peT_sb[:, e, ht, :],
                                    rhs=pf_sb[:, ht, lh * 512 : (lh + 1) * 512],
                                    start=first,
                                    stop=last,
                                )
                                first = False

                m2 = work.tile([TP, 2], F32, tag="m2")
                for lh in range(L // 512):
                    nc.vector.tensor_reduce(m2[:, lh : lh + 1], sc_ps[lh], axis=AX.X, op=OP.max)
                negm = work.tile([TP, 1], F32, tag="negm")
                nc.vector.tensor_reduce(negm, m2, axis=AX.X, op=OP.max, negate=True)
                p_sb = work.tile([TP, L], mm_dt, tag="p")
                s2 = work.tile([TP, 2], F32, tag="s2")
                for lh in range(L // 512):
                    nc.scalar.activation(
                        p_sb[:, lh * 512 : (lh + 1) * 512],
                        sc_ps[lh],
                        AF.Exp,
                        bias=negm,
                        scale=1.0,
                        accum_out=s2[:, lh : lh + 1],
                    )
                ssum = work.tile([TP, 1], F32, tag="ssum")
                nc.vector.tensor_reduce(ssum, s2, axis=AX.X, op=OP.add)
                rinv = mid.tile([TP, 1], F32, tag="rinv")
                nc.vector.reciprocal(rinv, ssum)

                pT_sb = mid.tile([128, LT, TP], mm_dt, tag="pT")
                tp_ps = ps_tp.tile([128, LT, TP], mm_dt, tag="tp")
                for lt in range(LT):
                    nc.tensor.transpose(tp_ps[:, lt, :], p_sb[:, lt * 128 : (lt + 1) * 128], ident[:TP, :TP])
                half = LT // 2
                nc.vector.tensor_copy(pT_sb[:, :half, :], tp_ps[:, :half, :])
                nc.scalar.copy(pT_sb[:, half:, :], tp_ps[:, half:, :])
                return pT_sb, rinv

            def back_mm2(b, state):
                """mm2 + rinv-scaled copy out of PSUM."""
                of_sb, (pT_sb, rinv) = state
                ctx_ps = ps_ctx.tile([TP, H], F32, tag="ctx")
                for lt in range(LT):
                    nc.tensor.matmul(
                        ctx_ps,
                        lhsT=pT_sb[:, lt, :],
                        rhs=of_sb[:, lt, :],
                        start=(lt == 0),
                        stop=(lt == LT - 1),
                    )
                ctx_sb = work.tile([TP, H], mm_dt, tag="ctx_sb")
                nc.vector.tensor_scalar_mul(ctx_sb, ctx_ps[:], rinv)
                return ctx_sb

            def back_tp(b, ctx_sb):
                """ctx transpose + copy to SBUF."""
                cT_sb = work.tile([128, HT, TP], mm_dt, tag="cT")
                tp_ps = ps_tp.tile([128, HT, TP], mm_dt, tag="tp")
                for ht in range(HT):
                    nc.tensor.transpose(tp_ps[:, ht, :], ctx_sb[:, ht * 128 : (ht + 1) * 128], ident[:TP, :TP])
                nc.scalar.copy(cT_sb, tp_ps)
                return cT_sb

            def back_mm3(b, cT_sb):
                """mm3 + bias + store."""
                o_ps = ps_tp.tile([C, TP], F32, tag="tp")
                for ht in range(HT):
                    nc.tensor.matmul(
                        o_ps,
                        lhsT=wgT_sb[:, ht, :],
                        rhs=cT_sb[:, ht, :],
                        start=(ht == 0),
                        stop=(ht == HT - 1),
                    )
                out_sb = work.tile([C, T], mm_dt, tag="out_sb")
                nc.vector.tensor_scalar_add(out_sb, o_ps[:, :T], bg_sb)
                nc.gpsimd.dma_start(outT[b], out_sb)

            for _rep in range(repeats):
                state = {}
                ofs = {}
                assert phased, "only the phased structure is maintained"
                # all pf loads + front halves first; of loads issued after
                # the last pf so pf(7) lands early; back halves trail,
                # software-pipelined (mm2 of b+1 hides b's scale/copy hops).
                for b in range(BPC):
                    state[b] = front_half(b)
                if _rep == 0:
                    nc.sync.dma_start(wgT_sb, wgT.rearrange("(ht p) c -> p ht c", p=128))
                    nc.sync.dma_start(bg_sb, bg)
                for b in range(BPC):
                    ofs[b] = load_of(b)
                ctxs = {}
                cts = {}
                for i in range(BPC + 2):
                    if i < BPC:
                        ctxs[i] = back_mm2(i, (ofs.pop(i), state.pop(i)))
                    if 1 <= i <= BPC:
                        cts[i - 1] = back_tp(i - 1, ctxs.pop(i - 1))
                    if i >= 2:
                        back_mm3(i - 2, cts.pop(i - 2))

    nc.compile()
    return nc


_NC = None


def _get_nc():
    global _NC
    if _NC is None:
        _NC = build_nc()
    return _NC


def make_in_maps(position_fmap, origin_fmap, pos_emb, W_gen, b_gen, np_dt=NP_DT, pe_terms=PE_TERMS):
    """Host-side sharding + layout prep. Returns list of per-core input dicts."""
    pf = np.asarray(position_fmap, dtype=np.float32)
    of = np.asarray(origin_fmap, dtype=np.float32)
    pe = np.asarray(pos_emb, dtype=np.float32)
    wg = np.asarray(W_gen, dtype=np.float32)
    bgv = np.asarray(b_gen, dtype=np.float32)

    # [B, L, H] -> [B, H, L] -> [B, 128, HT, L]  (partition-major, h = ht*128 + p)
    pfT = np.ascontiguousarray(
        pf.transpose(0, 2, 1).reshape(B, HT, 128, L).transpose(0, 2, 1, 3)
    ).astype(np_dt)
    # [B, L, H] -> [B, 128, LT, H]  (partition-major tiling, l = lt*128 + p)
    of_c = np.ascontiguousarray(
        of.reshape(B, LT, 128, H).transpose(0, 2, 1, 3)
    ).astype(np_dt)

    peT_f32 = np.zeros((H, TP), dtype=np.float32)
    peT_f32[:, :T] = pe.T
    terms = []
    resid = peT_f32
    for _ in range(pe_terms):
        t = resid.astype(np_dt)
        terms.append(t)
        resid = resid - t.astype(np.float32)
    peT = np.ascontiguousarray(np.stack(terms, axis=0))  # [pe_terms, H, TP]

    wgT = np.ascontiguousarray(wg.T).astype(np_dt)
    bg2 = np.ascontiguousarray(bgv.reshape(C, 1)).astype(np.float32)

    in_maps = []
    for i in range(NCORES):
        sl = slice(i * BPC, (i + 1) * BPC)
        in_maps.append(
            {
                "pfT": pfT[sl],
                "of": of_c[sl],
                "peT": peT,
                "wgT": wgT,
                "bg": bg2,
            }
        )
    return in_maps


def kernel(position_fmap, origin_fmap, pos_emb, W_gen, b_gen):
    nc = _get_nc()
    in_maps = make_in_maps(position_fmap, origin_fmap, pos_emb, W_gen, b_gen)
    res = run_bass_kernel_spmd(nc, in_maps, core_ids=list(range(NCORES)))
    outs = [r["outT"] for r in res.results]  # each [BPC, C, T]
    out = np.concatenate(outs, axis=0)  # [B, C, T]
    return np.ascontiguousarray(out.transpose(0, 2, 1)).astype(np.float32)

